# revision 2
# baseline (speedup 1.0000x reference)
"""Trainium2 Bass kernel for single-head attention (B=8, N=3136, C=147, D=64).

Sharding: data-parallel over batch across 8 NeuronCores (1 batch element/core).

Device runs only the O(N^2) attention core (S = q@k^T, exp, P@V); QKV
projections and the epilogue (softmax normalization, W_proj, bias,
v-residual) run on the host in fp32.

v2: 2x ROW-TILED PE (64x128 array mode). D=64 means the contraction for
S^T is only 64 partitions, and PV's K=128 contraction splits into two
64-row halves. Row tiles (0,0) and (64,0) run CONCURRENTLY on the PE
(microbenchmarked 1.83x vs serial K=128), so:
  - S^T: each slot computes TWO j-tiles at once: tile A (array rows
    0-63) does even j-tiles from kT2[0:64,:], tile B (rows 64-127) does
    odd j-tiles from kT2[64:128,:]; both stream the same q columns from
    their SBUF partition half. kT is zero-padded to 3200 cols so all 25
    j-tiles are 128 wide (uniform (64,128) tile mode, no PE drains;
    padded columns give exp(0)=1 times zero-padded v rows = 0).
  - PV: each j-tile's K=128 contraction is split: rows 0-63 accumulate
    into PSUM bank o[.,0,.], rows 64-127 into o[.,1,.]; the two partial
    outputs are summed during evacuation (ACT copies half B to SBUF,
    DVE adds half A to it).
With the PE halved, the exp becomes the bottleneck: ACT (1 elem/cyc @
1.2GHz) and DVE (1 elem/cyc @ 0.96GHz, Schraudolph int16 bit trick in a
single tensor_scalar) split the 13 exp units per chunk ~half/half.

i-chunks are 512 wide (6 full + one 64-wide tail chunk so the final
pipeline drain is short). st tiles keep a 512 inner stride so each
row-tiled matmul output stays inside one 2KB PSUM bank.
PSUM: st pairs 2 banks x 2 bufs + o pairs 2 banks x 2 bufs = 8 banks.
"""
import sys

for _p in ("/opt/trn_rl_repo",):
    if _p not in sys.path:
        sys.path.append(_p)

import numpy as np
import ml_dtypes
from contextlib import ExitStack

import concourse.bass as bass
import concourse.bacc as bacc
import concourse.tile as tile
from concourse import mybir
from concourse.bass_utils import run_bass_kernel_spmd

P = 128
SEQ = 3136        # N
CH = 147          # C
D = 64            # head dim
SCALE = D ** -0.5
NT = 25           # j-tiles (kT zero-padded 3136 -> 3200 = 25*128)
KPAD = NT * P     # 3200
IC = 512          # i-chunk width: 6*512 + 64
F32 = mybir.dt.float32
BF = mybir.dt.bfloat16
I16 = mybir.dt.int16
EXP = mybir.ActivationFunctionType.Exp
CPY = mybir.ActivationFunctionType.Copy
ADD = mybir.AluOpType.add

# Schraudolph constants: i16 = round(s * EA + EB); bits read as bf16 give
# ~e^s * (1 + eps(frac)), EB calibrated so E[eps] ~= 0.
EA = 128.0 * 1.4426950408889634
EB = 127.0 * 128.0 - 7.37

# exp engine assignment per slot (13 slots: 12 dual j-tile + 1 single):
# DVE takes these; ACT takes the rest + the o_B evac copy.
DVE_SLOTS = frozenset({0, 2, 4, 6, 8, 10})

_cache = {}


def _ichunks():
    out = []
    i0 = 0
    while i0 < SEQ:
        out.append((i0, min(IC, SEQ - i0)))
        i0 += IC
    return out


def build():
    nc = bacc.Bacc("TRN2", target_bir_lowering=False, debug=False, num_devices=8)
    qT2d = nc.declare_dram_parameter("qT2", [P, SEQ], BF, isOutput=False)
    kT2d = nc.declare_dram_parameter("kT2", [P, KPAD], BF, isOutput=False)
    v_aug = nc.declare_dram_parameter("v_aug", [P, NT, D + 1], BF, isOutput=False)
    chunks = _ichunks()
    oT = nc.declare_dram_parameter("oT", [len(chunks), D + 1, IC], F32,
                                   isOutput=True)

    with ExitStack() as ctx:
        tc = ctx.enter_context(tile.TileContext(nc))
        singles = ctx.enter_context(tc.tile_pool(name="singles", bufs=1))

        qT2 = singles.tile([P, SEQ], BF)   # qT duplicated in both halves
        kT2 = singles.tile([P, KPAD], BF)  # kT duplicated in both halves
        va = singles.tile([P, NT, D + 1], BF)
        # Input DMAs: first S^T slots need kT front + q chunk0; first PVs
        # need the early va tiles. Spread across queues so the critical
        # front pieces land first.
        nc.scalar.dma_start(out=kT2[:, 0:512], in_=kT2d[:, 0:512])
        nc.gpsimd.dma_start(out=va[:, 0:8, :], in_=v_aug[:, 0:8, :])
        nc.scalar.dma_start(out=qT2[:, 0:IC], in_=qT2d[:, 0:IC])
        nc.scalar.dma_start(out=kT2[:, 512:1536], in_=kT2d[:, 512:1536])
        nc.gpsimd.dma_start(out=kT2[:, 1536:KPAD], in_=kT2d[:, 1536:KPAD])
        nc.gpsimd.dma_start(out=va[:, 8:NT, :], in_=v_aug[:, 8:NT, :])
        for (n0, csz) in chunks[1:]:
            nc.sync.dma_start(out=qT2[:, n0:n0 + csz],
                              in_=qT2d[:, n0:n0 + csz])

        # --- HAM pre-warm + ACT exp-table preload, overlapping input DMA.
        # Junk init on GpSimd (free early); warm matmuls are row-tiled so
        # the PE never changes tile mode (no drains).
        with ExitStack() as wctx:
            warm_ps = wctx.enter_context(
                tc.tile_pool(name="warm_ps", bufs=2, space="PSUM"))
            junk_w = singles.tile([P, P], BF)
            junk_x = singles.tile([P, 256], BF)
            junk_e = singles.tile([P, 8], F32)
            junk_p = singles.tile([P, 8], BF)
            nc.gpsimd.memset(junk_w, 0.5)
            nc.gpsimd.memset(junk_x, 0.5)
            nc.gpsimd.memset(junk_e, 0.5)
            nc.scalar.activation(junk_p, junk_e, EXP)
            for i in range(16):
                wp = warm_ps.tile([P, 2, 512], F32, name="warm")
                nc.tensor.matmul(wp[:, 0, 0:256], junk_w[0:64, :],
                                 junk_x[0:64, :], start=True, stop=True,
                                 tile_position=(0, 0))
                nc.tensor.matmul(wp[:, 1, 0:256], junk_w[64:128, :],
                                 junk_x[64:128, :], start=True, stop=True,
                                 tile_position=(64, 0))

        # ---------------- attention ----------------
        with ExitStack() as cctx:
            st_ps = cctx.enter_context(tc.tile_pool(name="st_ps", bufs=2, space="PSUM"))
            o_ps_pool = cctx.enter_context(tc.tile_pool(name="o_ps", bufs=2, space="PSUM"))
            p_pool = cctx.enter_context(tc.tile_pool(name="p_sb", bufs=7))
            o_sb_pool = cctx.enter_context(tc.tile_pool(name="o_sb", bufs=2))
            ob_sb_pool = cctx.enter_context(tc.tile_pool(name="ob_sb", bufs=2))
            nslots = (NT + 1) // 2    # 13: 12 dual + 1 single

            def emit_pv(o_ps, p, pt, icsz):
                # row-split PV for the slot's j-tile(s): rows 0-63 -> bank
                # o[.,0,.], rows 64-127 -> bank o[.,1,.]
                for s in (0, 1):
                    jt = 2 * pt + s
                    if jt >= NT:
                        break
                    nc.tensor.matmul(o_ps[:, 0, 0:icsz], va[0:64, jt, :],
                                     p[0:64, s, 0:icsz],
                                     start=(jt == 0), stop=(jt == NT - 1),
                                     tile_position=(0, 0))
                    nc.tensor.matmul(o_ps[:, 1, 0:icsz], va[64:128, jt, :],
                                     p[64:128, s, 0:icsz],
                                     start=(jt == 0), stop=(jt == NT - 1),
                                     tile_position=(64, 0))

            def emit_evac(o_ps, ci, icsz):
                ob = ob_sb_pool.tile([D + 1, IC], F32, name="ob")
                osb = o_sb_pool.tile([D + 1, IC], F32, name="osb")
                nc.scalar.copy(ob[:, 0:icsz], o_ps[:, 1, 0:icsz])
                nc.vector.scalar_tensor_tensor(
                    out=osb[:, 0:icsz], in0=o_ps[:, 0, 0:icsz], scalar=0.0,
                    in1=ob[:, 0:icsz], op0=ADD, op1=ADD)
                nc.gpsimd.dma_start(out=oT[ci, :, 0:icsz],
                                    in_=osb[:, 0:icsz])

            pend = []            # (p, pt, o_ps, icsz) PV trails by 2 slots
            pending_out = None   # (o_ps, chunk index, icsz)
            for ci, (i0, icsz) in enumerate(chunks):
                o_ps = o_ps_pool.tile([D + 1, 2, IC], F32, name="o")
                for pt in range(nslots):
                    jtA, jtB = 2 * pt, 2 * pt + 1
                    st = st_ps.tile([P, 2, 512], F32, name="st")
                    p = p_pool.tile([P, 2, IC], BF, name="p")
                    nc.tensor.matmul(
                        st[:, 0, 0:icsz],
                        kT2[0:64, jtA * P:(jtA + 1) * P],
                        qT2[0:64, i0:i0 + icsz],
                        start=True, stop=True, tile_position=(0, 0))
                    if jtB < NT:
                        nc.tensor.matmul(
                            st[:, 1, 0:icsz],
                            kT2[64:128, jtB * P:(jtB + 1) * P],
                            qT2[64:128, i0:i0 + icsz],
                            start=True, stop=True, tile_position=(64, 0))
                        if pt in DVE_SLOTS:
                            nc.vector.tensor_scalar(
                                out=p[:, :, 0:icsz].bitcast(I16),
                                in0=st[:, :, 0:icsz],
                                scalar1=EA, scalar2=EB,
                                op0=mybir.AluOpType.mult,
                                op1=mybir.AluOpType.add)
                        else:
                            nc.scalar.activation(p[:, :, 0:icsz],
                                                 st[:, :, 0:icsz], EXP)
                    else:
                        nc.scalar.activation(p[:, 0, 0:icsz],
                                             st[:, 0, 0:icsz], EXP)
                    pend.append((p, pt, o_ps, icsz))
                    if len(pend) > 2:
                        ep, ept, eo, eic = pend.pop(0)
                        emit_pv(eo, ep, ept, eic)
                        if ept == nslots - 1 and pending_out is not None:
                            emit_evac(*pending_out)
                            pending_out = None
                pending_out = (o_ps, ci, icsz)
            while pend:
                ep, ept, eo, eic = pend.pop(0)
                emit_pv(eo, ep, ept, eic)
            emit_evac(*pending_out)

    nc.compile()
    return nc


def prep_in_maps(x, W_qkv, W_proj, b_proj):
    """Host-side prep: per-core transposed/duplicated bf16 operand layouts."""
    B = x.shape[0]
    bf = ml_dtypes.bfloat16
    Wq = (W_qkv[:, 0:D] * SCALE).astype(np.float32)
    Wk = W_qkv[:, D:2 * D].astype(np.float32)
    Wv = W_qkv[:, 2 * D:3 * D].astype(np.float32)
    in_maps = []
    vs = []
    for b in range(B):
        xb = x[b].astype(np.float32)
        v = xb @ Wv                                  # [N, D] fp32
        vs.append(v)
        vpad = np.zeros((KPAD, D + 1), np.float32)
        vpad[0:SEQ, 0:D] = v
        vpad[0:SEQ, D] = 1.0
        va = np.ascontiguousarray(
            vpad.reshape(NT, P, D + 1).transpose(1, 0, 2)).astype(bf)
        qT = np.ascontiguousarray((xb @ Wq).T)       # [D, N], pre-scaled
        kTp = np.zeros((D, KPAD), np.float32)
        kTp[:, 0:SEQ] = (xb @ Wk).T
        in_maps.append({
            "qT2": np.concatenate([qT, qT], axis=0).astype(bf),
            "kT2": np.concatenate([kTp, kTp], axis=0).astype(bf),
            "v_aug": va,
        })
    return in_maps, vs


def postprocess(results, vs, W_proj, b_proj):
    B = len(vs)
    chunks = _ichunks()
    out = np.empty((B, SEQ, D), np.float32)
    Wp = W_proj.astype(np.float32)
    bp = b_proj.astype(np.float32)
    for b in range(B):
        oT = results[b]["oT"]                        # [NCHUNK, 65, IC]
        O = np.concatenate(
            [oT[ci, :, 0:csz] for ci, (_, csz) in enumerate(chunks)], axis=1)
        attn = (O[0:D] / O[D:D + 1]).T               # [N, D]
        out[b] = vs[b] + attn @ Wp + bp
    return out


def kernel(x, W_qkv, W_proj, b_proj):
    B = x.shape[0]
    if "nc" not in _cache:
        _cache["nc"] = build()
    nc = _cache["nc"]
    in_maps, vs = prep_in_maps(x, W_qkv, W_proj, b_proj)
    res = run_bass_kernel_spmd(nc, in_maps, core_ids=list(range(B)))
    return postprocess(res.results, vs, W_proj, b_proj)


if __name__ == "__main__":
    rng = np.random.default_rng(0)
    x = rng.standard_normal((8, SEQ, CH), dtype=np.float32)
    W_qkv = (rng.standard_normal((CH, 3 * D), dtype=np.float32) * CH ** -0.5)
    W_proj = (rng.standard_normal((D, D), dtype=np.float32) * D ** -0.5)
    b_proj = np.zeros(D, dtype=np.float32)
    out = kernel(x, W_qkv, W_proj, b_proj)
    print("out", out.shape, out.dtype)
